# revision 10
# baseline (speedup 1.0000x reference)
"""AttentionConv2d pooling kernel for 8 Trainium2 NeuronCores.

Math: the reference computes, per batch n:
    tok = x[n].reshape(D, L).T                      # [L, D]
    K   = tok @ k_w.T + k_b + pos                   # [L, DOUT]
    V   = tok @ v_w.T + v_b                         # [L, DOUT]
    s   = K @ query / sqrt(DOUT)                    # [L]
    a   = softmax(s)                                # [L]
    out = a @ V                                     # [DOUT]

which collapses (since sum(a) == 1) to:
    q'  = k_w.T @ query / sqrt(DOUT)                # [D]
    ps  = (posMLP(grid) @ query) / sqrt(DOUT)       # [L]   (fourier MLP)
    c   = (k_b + b2) @ query / sqrt(DOUT)           # scalar, via Exp bias
    s   = x[n].T @ q' + ps
    u   = exp(s + c)    (scores are O(5), no max-subtraction needed)
    w   = x[n] @ u / sum(u)                         # [D]
    out = w @ v_w.T + v_b                           # [DOUT]

Sharding: data-parallel over batch N (2 batches per core); the fourier-MLP
pos-score is sharded over L: each core computes only its own l-range and the
8 ranges are exchanged with an AllGather.

Rotation trick: the host rotates each core's x chunks so that chunk j holds
l-range (c+j)%8 (softmax sums are order-invariant). Chunk 0 is then the
core's OWN range; chunks 1-3 use locally computed bridge ranges (own+1..3)
to ride through the collective's ~40us end-to-end latency; chunks 4+ take
their pos row from the gathered [8, LSH] table via a one-hot selector
matmul whose selector matrix is per-core INPUT DATA (the compiled program
stays identical across cores).

Host precompute: every weight-derived constant that doesn't involve x is
folded on the host (q', w2q = w2.T@query/16, the exp-bias scalar c, w1T/16,
vwT, wrT, biases) and shipped as two contiguous [128, X] blocks -- the
HWDGE descgen for per-element rearranges like "(oh p) -> p oh" costs
3.7-7.2us PER LOAD on the sync sequencer and was the old critical path.
The x-dependent compute (34 GB of scores/exp/weighted reductions) is the
actual workload and stays on device.

bf16 pipeline: x is cast fp32->bf16 during the DMA itself (SWDGE casts at
full HBM rate, measured 361-370 GB/s), which halves SBUF traffic/footprint
and doubles the PE matmul rate (full-rate bf16, 216ns/512col measured, vs
half-rate fp32r). All reductions (PSUM accumulation, exp-sum, weighted-sum
accumulators) stay fp32, so the end-to-end error stays ~2e-3 << the 2e-2
tolerance. The whole x stream rides the gpsimd queue alone (queue-mixing
measurably hurts SDMA throughput: 367 -> 314-329 GB/s); every small
transfer rides sync. The per-core HBM floor is 33.5MB / ~365GB/s ~= 92us.

Scheduling discipline (engines execute their streams IN ORDER): the j0
proj runs at priority 3000 and its hidden/out tail at 2800; everything
else keeps emission order, with bridge chains interleaved between unit
chunks so the Act engine alternates Exp(c_k) -> Sin(r_k+1) -> Gelu(r_k+1)
in large groups (one 1.3us table load per switch) while units keep the
DMA/DVE pipe full. The collective trigger is gpsimd-only; it is emitted
after chunk-1's x descgens so the Q7 reaches it right as the pos_own store
lands and the pre-issued descriptors keep the SDMA engines busy across the
short wait.
"""

import contextlib
import ctypes
import sys
import types

import ml_dtypes
import numpy as np

# ---------------------------------------------------------------------------
# antenv.axon_hooks shim: the image lacks this module; bass_utils imports it
# to capture NTFF profiles when trace=True. Provide the ctypes equivalent.
# ---------------------------------------------------------------------------
if "antenv.axon_hooks" not in sys.modules:
    _HOOK_CACHE = []

    def _make_ntff_hook():
        try:
            lib = ctypes.CDLL("/opt/axon/libaxon_pjrt.so")
        except OSError:
            return None
        if not hasattr(lib, "axon_start_nrt_profile"):
            return None
        lib.axon_start_nrt_profile.argtypes = [
            ctypes.POINTER(ctypes.c_int64),
            ctypes.c_size_t,
        ]
        lib.axon_start_nrt_profile.restype = ctypes.c_int64
        lib.axon_stop_nrt_profile.argtypes = [ctypes.c_char_p]
        lib.axon_stop_nrt_profile.restype = ctypes.c_int64

        @contextlib.contextmanager
        def _hook(output_dir, device_ids):
            import jax

            jax.devices()
            if device_ids:
                ids = (ctypes.c_int64 * len(device_ids))(*device_ids)
                rc = lib.axon_start_nrt_profile(ids, len(device_ids))
            else:
                rc = lib.axon_start_nrt_profile(None, 0)
            if rc != 0:
                raise RuntimeError(f"axon_start_nrt_profile rc={rc}")
            try:
                yield
            finally:
                n = lib.axon_stop_nrt_profile(str(output_dir).encode())
                print(f"ntff profile: {n} file(s) written to {output_dir}")

        return _hook

    def get_axon_ntff_profile_hook():
        if not _HOOK_CACHE:
            _HOOK_CACHE.append(_make_ntff_hook())
        return _HOOK_CACHE[0]

    _mod = types.ModuleType("antenv.axon_hooks")
    _mod.get_axon_ntff_profile_hook = get_axon_ntff_profile_hook
    sys.modules["antenv.axon_hooks"] = _mod

import concourse.bass as bass  # noqa: E402
import concourse.mybir as mybir  # noqa: E402
import concourse.tile as tile  # noqa: E402
from concourse import bacc  # noqa: E402
from concourse.bass_utils import run_bass_kernel_spmd  # noqa: E402

# Problem shapes (hardcoded per spec).
N, D, H, W = 16, 256, 128, 128
L = H * W  # 16384
DOUT = 256
NCORES = 8
NB = N // NCORES  # batches per core = 2
LSH = L // NCORES  # pos-score shard per core = 2048
LC = 2048  # l-chunk for the main loop (== LSH)
NCHUNK = L // LC  # chunks per batch = 8
NBRIDGE = 2  # locally computed bridge ranges (chunks 1..NBRIDGE)

F32 = mybir.dt.float32
BF16 = mybir.dt.bfloat16
AF = mybir.ActivationFunctionType
OP = mybir.AluOpType

INV_SQRT_D = 1.0 / 16.0  # 1/sqrt(DOUT)
HALF_PI = float(np.pi / 2.0)

# wbf (bf16 [128, 1026]) column map
WB_W1T = 0  # + fh*256 + j           : w1T/16   [f%128, fh, j]
WB_QREP = 512  # + dh*128 + k        : q' replicated along free
WB_W2Q = 768  # + jh                 : w2.T @ query/16 columns
WB_WRT = 770  # + f  (rows 0-1 only) : Wr.T
WB_ONES = 898  # + k  (row 0 only)   : ones row
WB_COLS = 1026
# wtf (fp32 [128, 518]) column map
WT_C = 0  # exp-bias scalar c, replicated
WT_B1 = 1  # + jh : b1 columns
WT_VB = 3  # + oh : v_b columns
WT_HALFPI = 5
WT_VWT = 6  # + dh*256 + o : vwT  [d%128, dh, o]
WT_COLS = 518


def build_program():
    nc = bacc.Bacc(
        "TRN2",
        target_bir_lowering=False,
        debug=False,
        enable_asserts=True,
        num_devices=NCORES,
    )

    # Per-core DRAM I/O (all host-prepared; see make_in_maps).
    x_d = nc.dram_tensor("x_sh", [NB, D, L], F32, kind="ExternalInput").ap()
    gg_d = nc.dram_tensor(
        "gg", [2, NBRIDGE + 1, LSH], BF16, kind="ExternalInput"
    ).ap()
    sel_d = nc.dram_tensor(
        "sel", [NCORES, NCHUNK, 128], BF16, kind="ExternalInput"
    ).ap()
    wbf_d = nc.dram_tensor("wbf", [128, WB_COLS], BF16, kind="ExternalInput").ap()
    wtf_d = nc.dram_tensor("wtf", [128, WT_COLS], F32, kind="ExternalInput").ap()
    out_d = nc.dram_tensor("out", [NB, DOUT], F32, kind="ExternalOutput").ap()

    # collective bounce buffers (internal DRAM; output must be Shared)
    pos_in_d = nc.dram_tensor("pos_in", [1, LSH], BF16).ap()
    pos_gather_d = nc.dram_tensor(
        "pos_gather", [NCORES, LSH], BF16, addr_space="Shared"
    ).ap()

    with tile.TileContext(nc) as tc:
        with (
            tc.tile_pool(name="const", bufs=1) as cpool,
            tc.tile_pool(name="state", bufs=1) as spool,
        ):
            sel_sb = cpool.tile([NCORES, NCHUNK, 128], BF16)
            pos_all = cpool.tile([NCORES, LSH], BF16)  # gathered pos table
            # per-batch accumulator tiles so batch n's final reduction only
            # waits on batch n's last unit (tile-granular dependencies).
            # cols 0..5: chunks 0-2 (2 half-accums each); 6..10: chunks 3-7.
            sexp_n0 = spool.tile([128, 11], F32)
            sexp_n1 = spool.tile([128, 11], F32)
            wpart_n0 = spool.tile([128, 2, NCHUNK], F32)
            wpart_n1 = spool.tile([128, 2, NCHUNK], F32)
            sexp_by_n = (sexp_n0, sexp_n1)
            wpart_by_n = (wpart_n0, wpart_n1)

            with (
                tc.tile_pool(name="xp", bufs=11) as xpool,
                tc.tile_pool(name="up", bufs=2) as upool,
                tc.tile_pool(name="scr", bufs=2) as scrpool,
                tc.tile_pool(name="cs", bufs=2) as cspool,
                tc.tile_pool(name="htp", bufs=2) as htpool,
                tc.tile_pool(name="pre", bufs=1) as ppool,
            ):
                # ---- constant loads on sync (x has gpsimd to itself) ------
                # Contiguous [partitions, X] blocks: one cheap descgen each.
                gg_t = ppool.tile([2, NBRIDGE + 1, LSH], BF16, tag="gg")
                nc.sync.dma_start(gg_t[:], gg_d)
                wbf = ppool.tile([128, WB_COLS], BF16, tag="wbf")
                nc.sync.dma_start(wbf[:], wbf_d)
                wtf = ppool.tile([128, WT_COLS], F32, tag="wtf")
                nc.sync.dma_start(wtf[:], wtf_d)
                nc.sync.dma_start(sel_sb[:], sel_d)

                wrT = wbf[0:2, WB_WRT : WB_WRT + 128]
                ones_row = wbf[0:1, WB_ONES : WB_ONES + 128]
                c_rep = wtf[:, WT_C : WT_C + 1]
                halfpi = wtf[:, WT_HALFPI : WT_HALFPI + 1]

                def emit_unit(c8, n, pspool, pos_stat, pos_mov, wide):
                    """One (chunk, batch) unit: cast-DMA, scores, exp,
                    fused mul-reduce.

                    pos_stat/pos_mov give the stationary AP and moving-slice
                    factory for the pos-add matmul (local ones-broadcast for
                    chunks 0-3, one-hot selector over the gathered table
                    after). The kb/b2 constant enters via the Exp bias.
                    wide=True uses a single [128, 2048] PSUM tile + one Exp
                    (4 banks; only once psR has closed); wide=False splits
                    into two [128, 1024] halves so pos chains fit alongside.
                    """
                    sexp_sb = sexp_by_n[n]
                    wpart_sb = wpart_by_n[n]
                    x_n = x_d[n].rearrange("(dh p) l -> p dh l", p=128)
                    csl = slice(c8 * LC, (c8 + 1) * LC)
                    xt = xpool.tile([128, 2, LC], BF16, tag="x")
                    nc.gpsimd.dma_start(xt[:], x_n[:, :, csl])  # casts to bf16
                    u_t = upool.tile([128, LC], BF16, tag="u")
                    if wide:
                        ps = pspool.tile([128, LC], F32, tag="w")
                        pss = (ps[:, 0:1024], ps[:, 1024:2048])
                    else:
                        ps0 = pspool.tile([128, LC // 2], F32, tag="s")
                        ps1 = pspool.tile([128, LC // 2], F32, tag="s")
                        pss = (ps0, ps1)
                    for dh in range(2):  # one stationary per dh group
                        q_st = wbf[:, WB_QREP + dh * 128 : WB_QREP + (dh + 1) * 128]
                        for hs in range(2):
                            for s2 in range(2):
                                sl = slice(
                                    hs * 1024 + s2 * 512,
                                    hs * 1024 + (s2 + 1) * 512,
                                )
                                nc.tensor.matmul(
                                    pss[hs][:, s2 * 512 : (s2 + 1) * 512],
                                    q_st,
                                    xt[:, dh, sl],
                                    start=(dh == 0),
                                    stop=False,
                                )
                    for hs in range(2):  # pos stationary closes all groups
                        for s2 in range(2):
                            lo = hs * 1024 + s2 * 512
                            nc.tensor.matmul(
                                pss[hs][:, s2 * 512 : (s2 + 1) * 512],
                                pos_stat,
                                pos_mov(lo),
                                start=False,
                                stop=True,
                            )
                    if wide:
                        nc.scalar.activation(
                            u_t[:], ps[:], AF.Exp,
                            bias=c_rep,
                            accum_out=sexp_sb[:, 3 + c8 : 4 + c8],
                        )
                    else:
                        for hs in range(2):
                            nc.scalar.activation(
                                u_t[:, hs * 1024 : (hs + 1) * 1024], pss[hs][:],
                                AF.Exp,
                                bias=c_rep,
                                accum_out=sexp_sb[
                                    :, 2 * c8 + hs : 2 * c8 + hs + 1
                                ],
                            )
                    scr = scrpool.tile([128, LC], BF16, tag="scr")
                    for dh in range(2):
                        nc.vector.affine_mul_reduce(
                            out=scr[:],
                            accum_out=wpart_sb[:, dh, c8 : c8 + 1],
                            in0=xt[:, dh, :],
                            in1=u_t[:],
                            scale=1.0,
                            bias=0.0,
                        )

                # ---- pos-MLP helpers ------------------------------------
                def emit_pos_proj(gg_r):
                    """proj + cos/sin for one l-range; returns (cos, sin)."""
                    HB = LSH // 2  # 1024: sub-range granularity
                    cos_sb = cspool.tile([128, LSH], BF16, tag="cos")
                    sin_sb = cspool.tile([128, LSH], BF16, tag="sin")
                    ps_rb0 = psR.tile([128, HB], F32, tag="rb")
                    ps_rb1 = psR.tile([128, HB], F32, tag="rb")
                    ps_rb = (ps_rb0, ps_rb1)
                    for sb2 in range(2):
                        for s in range(2):
                            sl = slice(
                                sb2 * HB + s * 512, sb2 * HB + (s + 1) * 512
                            )
                            nc.tensor.matmul(
                                ps_rb[sb2][:, s * 512 : (s + 1) * 512],
                                wrT, gg_r[:, sl],
                                start=True, stop=True,
                            )
                    for sb2 in range(2):
                        hsl = slice(sb2 * HB, (sb2 + 1) * HB)
                        nc.scalar.activation(
                            cos_sb[:, hsl], ps_rb[sb2][:], AF.Sin,
                            bias=halfpi,
                        )
                        nc.scalar.activation(sin_sb[:, hsl], ps_rb[sb2][:], AF.Sin)
                    return cos_sb, sin_sb

                def emit_pos_hidden(cos_sb, sin_sb):
                    """hidden matmuls + gelu for one l-range -> hT tile."""
                    HB = LSH // 2
                    hTb = htpool.tile([128, 2, LSH], BF16, tag="hT")
                    for jh in range(2):
                        ps_h0 = psR.tile([128, HB], F32, tag="rb")
                        ps_h1 = psR.tile([128, HB], F32, tag="rb")
                        ps_h = (ps_h0, ps_h1)
                        for fh, src_sb in ((0, cos_sb), (1, sin_sb)):
                            w1_st = wbf[
                                :,
                                WB_W1T + fh * 256 + jh * 128 :
                                WB_W1T + fh * 256 + (jh + 1) * 128,
                            ]
                            for sb2 in range(2):
                                for s in range(2):
                                    sl = slice(
                                        sb2 * HB + s * 512,
                                        sb2 * HB + (s + 1) * 512,
                                    )
                                    nc.tensor.matmul(
                                        ps_h[sb2][:, s * 512 : (s + 1) * 512],
                                        w1_st,
                                        src_sb[:, sl],
                                        start=(fh == 0),
                                        stop=(fh == 1),
                                    )
                        for sb2 in range(2):
                            hsl = slice(sb2 * HB, (sb2 + 1) * HB)
                            nc.scalar.activation(
                                hTb[:, jh, hsl], ps_h[sb2][:],
                                AF.Gelu_apprx_tanh,
                                bias=wtf[:, WT_B1 + jh : WT_B1 + jh + 1],
                            )
                    return hTb

                def emit_pos_out(hTb, pos_row):
                    """output row for one l-range (w2q dot along hidden)."""
                    HB = LSH // 2
                    ps_pos0 = psR.tile([128, HB], F32, tag="rb")
                    ps_pos1 = psR.tile([128, HB], F32, tag="rb")
                    ps_pos = (ps_pos0, ps_pos1)
                    for jh in range(2):
                        for sb2 in range(2):
                            for s in range(2):
                                sl = slice(
                                    sb2 * HB + s * 512, sb2 * HB + (s + 1) * 512
                                )
                                nc.tensor.matmul(
                                    ps_pos[sb2][0:1, s * 512 : (s + 1) * 512],
                                    wbf[:, WB_W2Q + jh : WB_W2Q + jh + 1],
                                    hTb[:, jh, sl],
                                    start=(jh == 0),
                                    stop=(jh == 1),
                                )
                    # raw pos row (the kb/b2 constant is applied as Exp bias)
                    for sb2 in range(2):
                        nc.vector.tensor_copy(
                            pos_row[0:1, sb2 * HB : (sb2 + 1) * HB],
                            ps_pos[sb2][0:1, :],
                        )

                with (
                    tc.tile_pool(name="psR", bufs=2, space="PSUM") as psR,
                    tc.tile_pool(name="psM", bufs=2, space="PSUM") as psM,
                ):
                    # j0 chain at high priority: it gates the collective
                    # dispatch, which is the kernel's longest latency.
                    with tc.high_priority(offset=3000):
                        cos0, sin0 = emit_pos_proj(gg_t[:, 0, :])
                    pos_own = ppool.tile([1, LC], BF16, tag="possh")
                    with tc.high_priority(offset=2800):
                        hT0 = emit_pos_hidden(cos0, sin0)
                        emit_pos_out(hT0, pos_own)
                    # store on sync at DEFAULT priority: it must stay behind
                    # the constant loads in the sync FIFO (its pos_own wait
                    # would otherwise block them).
                    nc.sync.dma_start(pos_in_d, pos_own[:])

                    # Bridge chains interleave BETWEEN unit chunks so the
                    # pos row r_k is always emitted (and computed) before
                    # the chunk-k units that consume it, while the Act
                    # engine alternates Exp / Sin / Gelu in large groups.
                    pos_rows = [pos_own]
                    for r in range(1, NBRIDGE + 1):
                        pos_rows.append(
                            ppool.tile(
                                [1, LC], BF16, tag=f"posr{r}", name=f"posr{r}"
                            )
                        )

                    # both bridge projs back-to-back (Act: 8 Sins, 1 table
                    # load), then chunk-0 units, then both hidden stages
                    # (8 Gelus, 1 load), so the Act stream is
                    # Sin x8 -> Exp(c0) -> Gelu x8 -> Exp(c1) -> Exp(c2)...
                    cs1 = emit_pos_proj(gg_t[:, 1, :])
                    cs2 = emit_pos_proj(gg_t[:, 2, :])
                    for n in range(NB):
                        emit_unit(
                            0, n, psM,
                            ones_row,
                            lambda lo: pos_own[0:1, lo : lo + 512],
                            wide=False,
                        )
                    hT1 = emit_pos_hidden(*cs1)
                    hT2 = emit_pos_hidden(*cs2)
                    emit_pos_out(hT1, pos_rows[1])
                    emit_pos_out(hT2, pos_rows[2])
                    for n in range(NB):
                        emit_unit(
                            1, n, psM,
                            ones_row,
                            lambda lo: pos_rows[1][0:1, lo : lo + 512],
                            wide=False,
                        )
                    # the collective trigger lives here in gpsimd program
                    # order: the pre-issued x descriptors keep the SDMA
                    # engines busy while the Q7 waits out the pos_in store.
                    nc.gpsimd.collective_compute(
                        "AllGather",
                        OP.bypass,
                        replica_groups=[list(range(NCORES))],
                        ins=[pos_in_d],
                        outs=[pos_gather_d],
                    )
                    for n in range(NB):
                        emit_unit(
                            2, n, psM,
                            ones_row,
                            lambda lo: pos_rows[2][0:1, lo : lo + 512],
                            wide=False,
                        )

                # ---- remaining chunks select from the gathered pos table --
                nc.sync.dma_start(pos_all[:], pos_gather_d)

                def emit_final(n, pspool):
                    """normalize + V projection + store for one batch."""
                    s_col = fpool.tile([128, 1], F32, tag="scol")
                    nc.vector.tensor_reduce(
                        s_col[:], sexp_by_n[n][:], mybir.AxisListType.X, OP.add
                    )
                    srec = fpool.tile([128, 1], F32, tag="srec")
                    nc.vector.reciprocal(srec[:], s_col[:])
                    wn = fpool.tile([128, 2], F32, tag="wn")
                    for dh in range(2):
                        wsum = fpool.tile([128, 1], F32, tag="wsum")
                        nc.vector.tensor_reduce(
                            wsum[:], wpart_by_n[n][:, dh, :],
                            mybir.AxisListType.X, OP.add,
                        )
                        nc.vector.tensor_scalar_mul(
                            wn[:, dh : dh + 1], wsum[:], srec[:]
                        )
                    for oh in range(2):
                        ps_ot = pspool.tile([128, LC], F32, tag="w")
                        ps_o = ps_ot[:, 0:1]
                        for dh in range(2):
                            nc.tensor.matmul(
                                ps_o,
                                wtf[
                                    :,
                                    WT_VWT + dh * 256 + oh * 128 :
                                    WT_VWT + dh * 256 + (oh + 1) * 128,
                                ],
                                wn[:, dh : dh + 1],
                                start=(dh == 0),
                                stop=(dh == 1),
                            )
                        o_sb = fpool.tile([128, 1], F32, tag="osb")
                        nc.vector.tensor_scalar_add(
                            o_sb[:], ps_o, wtf[:, WT_VB + oh : WT_VB + oh + 1]
                        )
                        nc.sync.dma_start(
                            out_d[n : n + 1, oh * 128 : (oh + 1) * 128], o_sb[:]
                        )

                with (
                    tc.tile_pool(name="psW", bufs=2, space="PSUM") as psW,
                    tc.tile_pool(name="fin", bufs=4) as fpool,
                ):
                    for c8 in range(NBRIDGE + 1, NCHUNK):
                        for n in range(NB):
                            emit_unit(
                                c8, n, psW,
                                sel_sb[:, c8, :],
                                lambda lo: pos_all[:, lo : lo + 512],
                                wide=True,
                            )
                            # batch n's accumulators are complete after its
                            # last chunk: finish it while the other drains
                            if c8 == NCHUNK - 1:
                                emit_final(n, psW)

    nc.compile()
    return nc


_NC_CACHE = []


def _get_nc():
    if not _NC_CACHE:
        _NC_CACHE.append(build_program())
    return _NC_CACHE[0]


def _grid_rows():
    """[gy; gx] rows of the normalized meshgrid, flattened to length L."""
    ys = np.linspace(-1.0, 1.0, H, dtype=np.float64)
    xs = np.linspace(-1.0, 1.0, W, dtype=np.float64)
    gy = np.repeat(ys, W)
    gx = np.tile(xs, H)
    return np.stack([gy, gx]).astype(np.float32)  # [2, L]


def _prep_consts(inputs):
    """Host-side fold of every weight-derived constant (fp64 accuracy)."""
    query = np.asarray(inputs["query"], dtype=np.float64)
    k_w = np.asarray(inputs["k_w"], dtype=np.float64)
    k_b = np.asarray(inputs["k_b"], dtype=np.float64)
    v_w = np.asarray(inputs["v_w"], dtype=np.float64)
    v_b = np.asarray(inputs["v_b"], dtype=np.float64)
    Wr = np.asarray(inputs["Wr"], dtype=np.float64)
    w1 = np.asarray(inputs["w1"], dtype=np.float64)
    b1 = np.asarray(inputs["b1"], dtype=np.float64)
    w2 = np.asarray(inputs["w2"], dtype=np.float64)
    b2 = np.asarray(inputs["b2"], dtype=np.float64)

    qs = query * INV_SQRT_D
    qp = k_w.T @ qs          # [D]   scores stationary
    w2q = w2.T @ qs          # [DOUT] pos-out stationary
    c = float((k_b + b2) @ qs)  # exp bias

    p = np.arange(128)
    wbf = np.zeros((128, WB_COLS), dtype=np.float32)
    for fh in range(2):
        # w1T[p, fh, j] = w1[j, fh*128+p] / 16
        wbf[:, WB_W1T + fh * 256 : WB_W1T + (fh + 1) * 256] = (
            w1[:, fh * 128 + p].T * INV_SQRT_D
        )
    for dh in range(2):
        wbf[:, WB_QREP + dh * 128 : WB_QREP + (dh + 1) * 128] = qp[
            dh * 128 + p, None
        ]
    for jh in range(2):
        wbf[:, WB_W2Q + jh] = w2q[jh * 128 + p]
    wbf[0, WB_WRT : WB_WRT + 128] = Wr[:, 0]
    wbf[1, WB_WRT : WB_WRT + 128] = Wr[:, 1]
    wbf[0, WB_ONES : WB_ONES + 128] = 1.0

    wtf = np.zeros((128, WT_COLS), dtype=np.float32)
    wtf[:, WT_C] = c
    for jh in range(2):
        wtf[:, WT_B1 + jh] = b1[jh * 128 + p]
    for oh in range(2):
        wtf[:, WT_VB + oh] = v_b[oh * 128 + p]
    wtf[:, WT_HALFPI] = HALF_PI
    for dh in range(2):
        # vwT[p, dh, o] = v_w[o, dh*128+p]
        wtf[:, WT_VWT + dh * 256 : WT_VWT + (dh + 1) * 256] = v_w[
            :, dh * 128 + p
        ].T
    return wbf.astype(ml_dtypes.bfloat16), wtf


def make_in_maps(inputs):
    x = np.ascontiguousarray(inputs["x"], dtype=np.float32).reshape(N, D, L)
    gg = _grid_rows()
    wbf, wtf = _prep_consts(inputs)
    in_maps = []
    for c in range(NCORES):
        m = {"wbf": wbf, "wtf": wtf}
        # rotate the l-chunks so chunk j holds range (c+j)%NCHUNK
        xc = x[c * NB : (c + 1) * NB].reshape(NB, D, NCHUNK, LC)
        m["x_sh"] = np.ascontiguousarray(
            np.roll(xc, -c, axis=2).reshape(NB, D, L)
        )
        ggc = np.stack(
            [
                gg[:, r * LSH : (r + 1) * LSH]
                for r in [(c + j) % NCORES for j in range(NBRIDGE + 1)]
            ],
            axis=1,
        )  # [2, NBRIDGE+1, LSH]
        m["gg"] = np.ascontiguousarray(ggc).astype(ml_dtypes.bfloat16)
        sel = np.zeros((NCORES, NCHUNK, 128), dtype=np.float32)
        for j in range(NCHUNK):
            sel[(c + j) % NCHUNK, j, :] = 1.0
        m["sel"] = sel.astype(ml_dtypes.bfloat16)
        in_maps.append(m)
    return in_maps


def run(inputs, trace=False):
    nc = _get_nc()
    res = run_bass_kernel_spmd(
        nc, make_in_maps(inputs), core_ids=list(range(NCORES)), trace=trace
    )
    out = np.concatenate([res.results[c]["out"] for c in range(NCORES)], axis=0)
    return out.astype(np.float32), res


def kernel(**inputs) -> np.ndarray:
    out, _ = run(inputs, trace=False)
    return out


# revision 12
# speedup vs baseline: 1.1789x; 1.1789x over previous
"""AttentionConv2d pooling kernel for 8 Trainium2 NeuronCores.

Math: the reference computes, per batch n:
    tok = x[n].reshape(D, L).T                      # [L, D]
    K   = tok @ k_w.T + k_b + pos                   # [L, DOUT]
    V   = tok @ v_w.T + v_b                         # [L, DOUT]
    s   = K @ query / sqrt(DOUT)                    # [L]
    a   = softmax(s)                                # [L]
    out = a @ V                                     # [DOUT]

which collapses (since sum(a) == 1) to:
    q'  = k_w.T @ query / sqrt(DOUT)                # [D]
    ps  = (posMLP(grid) @ query) / sqrt(DOUT)       # [L]   (fourier MLP)
    c   = (k_b + b2) @ query / sqrt(DOUT)           # scalar, via Exp bias
    s   = x[n].T @ q' + ps
    u   = exp(s + c)    (scores are O(5), no max-subtraction needed)
    w   = x[n] @ u / sum(u)                         # [D]
    out = w @ v_w.T + v_b                           # [DOUT]

Sharding: data-parallel over batch N (2 batches per core); the fourier-MLP
pos-score is sharded over L: each core computes only its own l-range and the
8 ranges are exchanged with an AllGather.

Rotation trick: the host rotates each core's x chunks so that chunk j holds
l-range (c+j)%8 (softmax sums are order-invariant). Chunk 0 is then the
core's OWN range; chunks 1-3 use locally computed bridge ranges (own+1..3)
to ride through the collective's ~40us end-to-end latency; chunks 4+ take
their pos row from the gathered [8, LSH] table via a one-hot selector
matmul whose selector matrix is per-core INPUT DATA (the compiled program
stays identical across cores).

Host precompute: every weight-derived constant that doesn't involve x is
folded on the host (q', w2q = w2.T@query/16, the exp-bias scalar c, w1T/16,
vwT, wrT, biases) and shipped as two contiguous [128, X] blocks -- the
HWDGE descgen for per-element rearranges like "(oh p) -> p oh" costs
3.7-7.2us PER LOAD on the sync sequencer and was the old critical path.
The x-dependent compute (34 GB of scores/exp/weighted reductions) is the
actual workload and stays on device.

bf16 pipeline: x is cast fp32->bf16 during the DMA itself (SWDGE casts at
full HBM rate, measured 361-370 GB/s), which halves SBUF traffic/footprint
and doubles the PE matmul rate (full-rate bf16, 216ns/512col measured, vs
half-rate fp32r). All reductions (PSUM accumulation, exp-sum, weighted-sum
accumulators) stay fp32, so the end-to-end error stays ~2e-3 << the 2e-2
tolerance. The whole x stream rides the gpsimd queue alone (queue-mixing
measurably hurts SDMA throughput: 367 -> 314-329 GB/s); every small
transfer rides sync. The per-core HBM floor is 33.5MB / ~365GB/s ~= 92us.

Scheduling discipline (engines execute their streams IN ORDER): the j0
proj runs at priority 3000 and its hidden/out tail at 2800; everything
else keeps emission order, with bridge chains interleaved between unit
chunks so the Act engine alternates Exp(c_k) -> Sin(r_k+1) -> Gelu(r_k+1)
in large groups (one 1.3us table load per switch) while units keep the
DMA/DVE pipe full. The collective trigger is gpsimd-only; it is emitted
after chunk-1's x descgens so the Q7 reaches it right as the pos_own store
lands and the pre-issued descriptors keep the SDMA engines busy across the
short wait.
"""

import contextlib
import ctypes
import sys
import types

import ml_dtypes
import numpy as np

# ---------------------------------------------------------------------------
# antenv.axon_hooks shim: the image lacks this module; bass_utils imports it
# to capture NTFF profiles when trace=True. Provide the ctypes equivalent.
# ---------------------------------------------------------------------------
if "antenv.axon_hooks" not in sys.modules:
    _HOOK_CACHE = []

    def _make_ntff_hook():
        try:
            lib = ctypes.CDLL("/opt/axon/libaxon_pjrt.so")
        except OSError:
            return None
        if not hasattr(lib, "axon_start_nrt_profile"):
            return None
        lib.axon_start_nrt_profile.argtypes = [
            ctypes.POINTER(ctypes.c_int64),
            ctypes.c_size_t,
        ]
        lib.axon_start_nrt_profile.restype = ctypes.c_int64
        lib.axon_stop_nrt_profile.argtypes = [ctypes.c_char_p]
        lib.axon_stop_nrt_profile.restype = ctypes.c_int64

        @contextlib.contextmanager
        def _hook(output_dir, device_ids):
            import jax

            jax.devices()
            if device_ids:
                ids = (ctypes.c_int64 * len(device_ids))(*device_ids)
                rc = lib.axon_start_nrt_profile(ids, len(device_ids))
            else:
                rc = lib.axon_start_nrt_profile(None, 0)
            if rc != 0:
                raise RuntimeError(f"axon_start_nrt_profile rc={rc}")
            try:
                yield
            finally:
                n = lib.axon_stop_nrt_profile(str(output_dir).encode())
                print(f"ntff profile: {n} file(s) written to {output_dir}")

        return _hook

    def get_axon_ntff_profile_hook():
        if not _HOOK_CACHE:
            _HOOK_CACHE.append(_make_ntff_hook())
        return _HOOK_CACHE[0]

    _mod = types.ModuleType("antenv.axon_hooks")
    _mod.get_axon_ntff_profile_hook = get_axon_ntff_profile_hook
    sys.modules["antenv.axon_hooks"] = _mod

import concourse.bass as bass  # noqa: E402
import concourse.mybir as mybir  # noqa: E402
import concourse.tile as tile  # noqa: E402
from concourse import bacc  # noqa: E402
from concourse.bass_utils import run_bass_kernel_spmd  # noqa: E402

# Problem shapes (hardcoded per spec).
N, D, H, W = 16, 256, 128, 128
L = H * W  # 16384
DOUT = 256
NCORES = 8
NB = N // NCORES  # batches per core = 2
LSH = L // NCORES  # pos-score shard per core = 2048
LC = 2048  # l-chunk for the main loop (== LSH)
NCHUNK = L // LC  # chunks per batch = 8
NBRIDGE = 2  # locally computed bridge ranges (chunks 1..NBRIDGE)

F32 = mybir.dt.float32
BF16 = mybir.dt.bfloat16
AF = mybir.ActivationFunctionType
OP = mybir.AluOpType

INV_SQRT_D = 1.0 / 16.0  # 1/sqrt(DOUT)
HALF_PI = float(np.pi / 2.0)

# wbf (bf16 [128, 1026]) column map
WB_W1T = 0  # + fh*256 + j           : w1T/16   [f%128, fh, j]
WB_QREP = 512  # + dh*128 + k        : q' replicated along free
WB_W2Q = 768  # + jh                 : w2.T @ query/16 columns
WB_WRT = 770  # + f  (rows 0-1 only) : Wr.T
WB_ONES = 898  # + k  (row 0 only)   : ones row
WB_COLS = 1026
# wtf (fp32 [128, 518]) column map
WT_C = 0  # exp-bias scalar c, replicated
WT_B1 = 1  # + jh : b1 columns
WT_VB = 3  # + oh : v_b columns
WT_HALFPI = 5
WT_VWT = 6  # + dh*256 + o : vwT  [d%128, dh, o]
WT_COLS = 518


def build_program():
    nc = bacc.Bacc(
        "TRN2",
        target_bir_lowering=False,
        debug=False,
        enable_asserts=True,
        num_devices=NCORES,
    )

    # Per-core DRAM I/O (all host-prepared; see make_in_maps).
    x_d = nc.dram_tensor("x_sh", [NB, D, L], F32, kind="ExternalInput").ap()
    gg_d = nc.dram_tensor(
        "gg", [2, NBRIDGE + 1, LSH], BF16, kind="ExternalInput"
    ).ap()
    sel_d = nc.dram_tensor(
        "sel", [NCORES, NCHUNK, 128], BF16, kind="ExternalInput"
    ).ap()
    wbf_d = nc.dram_tensor("wbf", [128, WB_COLS], BF16, kind="ExternalInput").ap()
    wtf_d = nc.dram_tensor("wtf", [128, WT_COLS], F32, kind="ExternalInput").ap()
    out_d = nc.dram_tensor("out", [NB, DOUT], F32, kind="ExternalOutput").ap()

    # collective bounce buffers (internal DRAM; output must be Shared)
    pos_in_d = nc.dram_tensor("pos_in", [1, LSH], BF16).ap()
    pos_gather_d = nc.dram_tensor(
        "pos_gather", [NCORES, LSH], BF16, addr_space="Shared"
    ).ap()

    with tile.TileContext(nc) as tc:
        with (
            tc.tile_pool(name="const", bufs=1) as cpool,
            tc.tile_pool(name="state", bufs=1) as spool,
        ):
            sel_sb = cpool.tile([NCORES, NCHUNK, 128], BF16)
            pos_all = cpool.tile([NCORES, LSH], BF16)  # gathered pos table
            # per-batch accumulator tiles so batch n's final reduction only
            # waits on batch n's last unit (tile-granular dependencies).
            # cols 0..5: chunks 0-2 (2 half-accums each); 6..10: chunks 3-7.
            sexp_n0 = spool.tile([128, 11], F32)
            sexp_n1 = spool.tile([128, 11], F32)
            wpart_n0 = spool.tile([128, 2, NCHUNK], F32)
            wpart_n1 = spool.tile([128, 2, NCHUNK], F32)
            sexp_by_n = (sexp_n0, sexp_n1)
            wpart_by_n = (wpart_n0, wpart_n1)

            with (
                tc.tile_pool(name="xp", bufs=11) as xpool,
                tc.tile_pool(name="up", bufs=2) as upool,
                tc.tile_pool(name="scr", bufs=2) as scrpool,
                tc.tile_pool(name="cs", bufs=2) as cspool,
                tc.tile_pool(name="htp", bufs=2) as htpool,
                tc.tile_pool(name="pre", bufs=1) as ppool,
            ):
                # ---- constant loads on sync (x has gpsimd to itself) ------
                # Contiguous [partitions, X] blocks: one cheap descgen each.
                gg_t = ppool.tile([2, NBRIDGE + 1, LSH], BF16, tag="gg")
                nc.sync.dma_start(gg_t[:], gg_d)
                wbf = ppool.tile([128, WB_COLS], BF16, tag="wbf")
                nc.sync.dma_start(wbf[:], wbf_d)
                wtf = ppool.tile([128, WT_COLS], F32, tag="wtf")
                nc.sync.dma_start(wtf[:], wtf_d)
                nc.sync.dma_start(sel_sb[:], sel_d)

                wrT = wbf[0:2, WB_WRT : WB_WRT + 128]
                ones_row = wbf[0:1, WB_ONES : WB_ONES + 128]
                c_rep = wtf[:, WT_C : WT_C + 1]
                halfpi = wtf[:, WT_HALFPI : WT_HALFPI + 1]

                def emit_unit(c8, n, pspool, pos_stat, pos_mov, wide):
                    """One (chunk, batch) unit: cast-DMA, scores, exp,
                    fused mul-reduce.

                    pos_stat/pos_mov give the stationary AP and moving-slice
                    factory for the pos-add matmul (local ones-broadcast for
                    chunks 0-3, one-hot selector over the gathered table
                    after). The kb/b2 constant enters via the Exp bias.
                    wide=True uses a single [128, 2048] PSUM tile + one Exp
                    (4 banks; only once psR has closed); wide=False splits
                    into two [128, 1024] halves so pos chains fit alongside.
                    """
                    sexp_sb = sexp_by_n[n]
                    wpart_sb = wpart_by_n[n]
                    x_n = x_d[n].rearrange("(dh p) l -> p dh l", p=128)
                    csl = slice(c8 * LC, (c8 + 1) * LC)
                    xt = xpool.tile([128, 2, LC], BF16, tag="x")
                    nc.gpsimd.dma_start(xt[:], x_n[:, :, csl])  # casts to bf16
                    u_t = upool.tile([128, LC], BF16, tag="u")
                    if wide:
                        ps = pspool.tile([128, LC], F32, tag="w")
                        pss = (ps[:, 0:1024], ps[:, 1024:2048])
                    else:
                        ps0 = pspool.tile([128, LC // 2], F32, tag="s")
                        ps1 = pspool.tile([128, LC // 2], F32, tag="s")
                        pss = (ps0, ps1)
                    for dh in range(2):  # one stationary per dh group
                        q_st = wbf[:, WB_QREP + dh * 128 : WB_QREP + (dh + 1) * 128]
                        for hs in range(2):
                            for s2 in range(2):
                                sl = slice(
                                    hs * 1024 + s2 * 512,
                                    hs * 1024 + (s2 + 1) * 512,
                                )
                                nc.tensor.matmul(
                                    pss[hs][:, s2 * 512 : (s2 + 1) * 512],
                                    q_st,
                                    xt[:, dh, sl],
                                    start=(dh == 0),
                                    stop=False,
                                )
                    for hs in range(2):  # pos stationary closes all groups
                        for s2 in range(2):
                            lo = hs * 1024 + s2 * 512
                            nc.tensor.matmul(
                                pss[hs][:, s2 * 512 : (s2 + 1) * 512],
                                pos_stat,
                                pos_mov(lo),
                                start=False,
                                stop=True,
                            )
                    if wide:
                        nc.scalar.activation(
                            u_t[:], ps[:], AF.Exp,
                            bias=c_rep,
                            accum_out=sexp_sb[:, 3 + c8 : 4 + c8],
                        )
                    else:
                        for hs in range(2):
                            nc.scalar.activation(
                                u_t[:, hs * 1024 : (hs + 1) * 1024], pss[hs][:],
                                AF.Exp,
                                bias=c_rep,
                                accum_out=sexp_sb[
                                    :, 2 * c8 + hs : 2 * c8 + hs + 1
                                ],
                            )
                    scr = scrpool.tile([128, LC], BF16, tag="scr")
                    for dh in range(2):
                        nc.vector.affine_mul_reduce(
                            out=scr[:],
                            accum_out=wpart_sb[:, dh, c8 : c8 + 1],
                            in0=xt[:, dh, :],
                            in1=u_t[:],
                            scale=1.0,
                            bias=0.0,
                        )

                # ---- pos-MLP helpers ------------------------------------
                def emit_pos_proj(gg_r):
                    """proj + cos/sin for one l-range; returns (cos, sin)."""
                    HB = LSH // 2  # 1024: sub-range granularity
                    cos_sb = cspool.tile([128, LSH], BF16, tag="cos")
                    sin_sb = cspool.tile([128, LSH], BF16, tag="sin")
                    ps_rb0 = psR.tile([128, HB], F32, tag="rb")
                    ps_rb1 = psR.tile([128, HB], F32, tag="rb")
                    ps_rb = (ps_rb0, ps_rb1)
                    for sb2 in range(2):
                        for s in range(2):
                            sl = slice(
                                sb2 * HB + s * 512, sb2 * HB + (s + 1) * 512
                            )
                            nc.tensor.matmul(
                                ps_rb[sb2][:, s * 512 : (s + 1) * 512],
                                wrT, gg_r[:, sl],
                                start=True, stop=True,
                            )
                    for sb2 in range(2):
                        hsl = slice(sb2 * HB, (sb2 + 1) * HB)
                        nc.scalar.activation(
                            cos_sb[:, hsl], ps_rb[sb2][:], AF.Sin,
                            bias=halfpi,
                        )
                        nc.scalar.activation(sin_sb[:, hsl], ps_rb[sb2][:], AF.Sin)
                    return cos_sb, sin_sb

                def emit_pos_hidden(cos_sb, sin_sb):
                    """hidden matmuls + gelu for one l-range -> hT tile."""
                    HB = LSH // 2
                    hTb = htpool.tile([128, 2, LSH], BF16, tag="hT")
                    for jh in range(2):
                        ps_h0 = psR.tile([128, HB], F32, tag="rb")
                        ps_h1 = psR.tile([128, HB], F32, tag="rb")
                        ps_h = (ps_h0, ps_h1)
                        for fh, src_sb in ((0, cos_sb), (1, sin_sb)):
                            w1_st = wbf[
                                :,
                                WB_W1T + fh * 256 + jh * 128 :
                                WB_W1T + fh * 256 + (jh + 1) * 128,
                            ]
                            for sb2 in range(2):
                                for s in range(2):
                                    sl = slice(
                                        sb2 * HB + s * 512,
                                        sb2 * HB + (s + 1) * 512,
                                    )
                                    nc.tensor.matmul(
                                        ps_h[sb2][:, s * 512 : (s + 1) * 512],
                                        w1_st,
                                        src_sb[:, sl],
                                        start=(fh == 0),
                                        stop=(fh == 1),
                                    )
                        for sb2 in range(2):
                            hsl = slice(sb2 * HB, (sb2 + 1) * HB)
                            nc.scalar.activation(
                                hTb[:, jh, hsl], ps_h[sb2][:],
                                AF.Gelu_apprx_tanh,
                                bias=wtf[:, WT_B1 + jh : WT_B1 + jh + 1],
                            )
                    return hTb

                def emit_pos_out(hTb, pos_row):
                    """output row for one l-range (w2q dot along hidden)."""
                    HB = LSH // 2
                    ps_pos0 = psR.tile([128, HB], F32, tag="rb")
                    ps_pos1 = psR.tile([128, HB], F32, tag="rb")
                    ps_pos = (ps_pos0, ps_pos1)
                    for jh in range(2):
                        for sb2 in range(2):
                            for s in range(2):
                                sl = slice(
                                    sb2 * HB + s * 512, sb2 * HB + (s + 1) * 512
                                )
                                nc.tensor.matmul(
                                    ps_pos[sb2][0:1, s * 512 : (s + 1) * 512],
                                    wbf[:, WB_W2Q + jh : WB_W2Q + jh + 1],
                                    hTb[:, jh, sl],
                                    start=(jh == 0),
                                    stop=(jh == 1),
                                )
                    # raw pos row (the kb/b2 constant is applied as Exp bias)
                    for sb2 in range(2):
                        nc.vector.tensor_copy(
                            pos_row[0:1, sb2 * HB : (sb2 + 1) * HB],
                            ps_pos[sb2][0:1, :],
                        )

                with (
                    tc.tile_pool(name="psR", bufs=2, space="PSUM") as psR,
                    tc.tile_pool(name="psM", bufs=2, space="PSUM") as psM,
                ):
                    # j0 chain at high priority: it gates the collective
                    # dispatch, which is the kernel's longest latency.
                    with tc.high_priority(offset=3000):
                        cos0, sin0 = emit_pos_proj(gg_t[:, 0, :])
                    pos_own = ppool.tile([1, LC], BF16, tag="possh")
                    with tc.high_priority(offset=2800):
                        hT0 = emit_pos_hidden(cos0, sin0)
                        emit_pos_out(hT0, pos_own)
                    # store on sync at DEFAULT priority: it must stay behind
                    # the constant loads in the sync FIFO (its pos_own wait
                    # would otherwise block them).
                    nc.sync.dma_start(pos_in_d, pos_own[:])

                    # Bridge chains interleave BETWEEN unit chunks so the
                    # pos row r_k is always emitted (and computed) before
                    # the chunk-k units that consume it, while the Act
                    # engine alternates Exp / Sin / Gelu in large groups.
                    pos_rows = [pos_own]
                    for r in range(1, NBRIDGE + 1):
                        pos_rows.append(
                            ppool.tile(
                                [1, LC], BF16, tag=f"posr{r}", name=f"posr{r}"
                            )
                        )

                    # both bridge projs back-to-back (Act: 8 Sins, 1 table
                    # load), then chunk-0 units, then both hidden stages
                    # (8 Gelus, 1 load), so the Act stream is
                    # Sin x8 -> Exp(c0) -> Gelu x8 -> Exp(c1) -> Exp(c2)...
                    cs1 = emit_pos_proj(gg_t[:, 1, :])
                    cs2 = emit_pos_proj(gg_t[:, 2, :])
                    for n in range(NB):
                        emit_unit(
                            0, n, psM,
                            ones_row,
                            lambda lo: pos_own[0:1, lo : lo + 512],
                            wide=False,
                        )
                    hT1 = emit_pos_hidden(*cs1)
                    emit_pos_out(hT1, pos_rows[1])
                    hT2 = emit_pos_hidden(*cs2)
                    emit_pos_out(hT2, pos_rows[2])
                    for n in range(NB):
                        emit_unit(
                            1, n, psM,
                            ones_row,
                            lambda lo: pos_rows[1][0:1, lo : lo + 512],
                            wide=False,
                        )
                    # the collective trigger lives here in gpsimd program
                    # order: the pre-issued x descriptors keep the SDMA
                    # engines busy while the Q7 waits out the pos_in store.
                    nc.gpsimd.collective_compute(
                        "AllGather",
                        OP.bypass,
                        replica_groups=[list(range(NCORES))],
                        ins=[pos_in_d],
                        outs=[pos_gather_d],
                    )
                    for n in range(NB):
                        emit_unit(
                            2, n, psM,
                            ones_row,
                            lambda lo: pos_rows[2][0:1, lo : lo + 512],
                            wide=False,
                        )

                # ---- remaining chunks select from the gathered pos table --
                nc.sync.dma_start(pos_all[:], pos_gather_d)

                def emit_final(n, pspool):
                    """normalize + V projection + store for one batch."""
                    s_col = fpool.tile([128, 1], F32, tag="scol")
                    nc.vector.tensor_reduce(
                        s_col[:], sexp_by_n[n][:], mybir.AxisListType.X, OP.add
                    )
                    srec = fpool.tile([128, 1], F32, tag="srec")
                    nc.vector.reciprocal(srec[:], s_col[:])
                    wn = fpool.tile([128, 2], F32, tag="wn")
                    for dh in range(2):
                        wsum = fpool.tile([128, 1], F32, tag="wsum")
                        nc.vector.tensor_reduce(
                            wsum[:], wpart_by_n[n][:, dh, :],
                            mybir.AxisListType.X, OP.add,
                        )
                        nc.vector.tensor_scalar_mul(
                            wn[:, dh : dh + 1], wsum[:], srec[:]
                        )
                    for oh in range(2):
                        ps_ot = pspool.tile([128, LC], F32, tag="w")
                        ps_o = ps_ot[:, 0:1]
                        for dh in range(2):
                            nc.tensor.matmul(
                                ps_o,
                                wtf[
                                    :,
                                    WT_VWT + dh * 256 + oh * 128 :
                                    WT_VWT + dh * 256 + (oh + 1) * 128,
                                ],
                                wn[:, dh : dh + 1],
                                start=(dh == 0),
                                stop=(dh == 1),
                            )
                        o_sb = fpool.tile([128, 1], F32, tag="osb")
                        nc.vector.tensor_scalar_add(
                            o_sb[:], ps_o, wtf[:, WT_VB + oh : WT_VB + oh + 1]
                        )
                        nc.sync.dma_start(
                            out_d[n : n + 1, oh * 128 : (oh + 1) * 128], o_sb[:]
                        )

                with (
                    tc.tile_pool(name="psW", bufs=2, space="PSUM") as psW,
                    tc.tile_pool(name="fin", bufs=4) as fpool,
                ):
                    for c8 in range(NBRIDGE + 1, NCHUNK):
                        for n in range(NB):
                            emit_unit(
                                c8, n, psW,
                                sel_sb[:, c8, :],
                                lambda lo: pos_all[:, lo : lo + 512],
                                wide=True,
                            )
                    # finals strictly after the last unit: their PSUM tiles
                    # must not interpose in the psW ring ahead of c7's
                    # matmuls (the ring is allocation-ordered).
                    for n in range(NB):
                        emit_final(n, psW)

    nc.compile()
    return nc


_NC_CACHE = []


def _get_nc():
    if not _NC_CACHE:
        _NC_CACHE.append(build_program())
    return _NC_CACHE[0]


def _grid_rows():
    """[gy; gx] rows of the normalized meshgrid, flattened to length L."""
    ys = np.linspace(-1.0, 1.0, H, dtype=np.float64)
    xs = np.linspace(-1.0, 1.0, W, dtype=np.float64)
    gy = np.repeat(ys, W)
    gx = np.tile(xs, H)
    return np.stack([gy, gx]).astype(np.float32)  # [2, L]


def _prep_consts(inputs):
    """Host-side fold of every weight-derived constant (fp64 accuracy)."""
    query = np.asarray(inputs["query"], dtype=np.float64)
    k_w = np.asarray(inputs["k_w"], dtype=np.float64)
    k_b = np.asarray(inputs["k_b"], dtype=np.float64)
    v_w = np.asarray(inputs["v_w"], dtype=np.float64)
    v_b = np.asarray(inputs["v_b"], dtype=np.float64)
    Wr = np.asarray(inputs["Wr"], dtype=np.float64)
    w1 = np.asarray(inputs["w1"], dtype=np.float64)
    b1 = np.asarray(inputs["b1"], dtype=np.float64)
    w2 = np.asarray(inputs["w2"], dtype=np.float64)
    b2 = np.asarray(inputs["b2"], dtype=np.float64)

    qs = query * INV_SQRT_D
    qp = k_w.T @ qs          # [D]   scores stationary
    w2q = w2.T @ qs          # [DOUT] pos-out stationary
    c = float((k_b + b2) @ qs)  # exp bias

    p = np.arange(128)
    wbf = np.zeros((128, WB_COLS), dtype=np.float32)
    for fh in range(2):
        # w1T[p, fh, j] = w1[j, fh*128+p] / 16
        wbf[:, WB_W1T + fh * 256 : WB_W1T + (fh + 1) * 256] = (
            w1[:, fh * 128 + p].T * INV_SQRT_D
        )
    for dh in range(2):
        wbf[:, WB_QREP + dh * 128 : WB_QREP + (dh + 1) * 128] = qp[
            dh * 128 + p, None
        ]
    for jh in range(2):
        wbf[:, WB_W2Q + jh] = w2q[jh * 128 + p]
    wbf[0, WB_WRT : WB_WRT + 128] = Wr[:, 0]
    wbf[1, WB_WRT : WB_WRT + 128] = Wr[:, 1]
    wbf[0, WB_ONES : WB_ONES + 128] = 1.0

    wtf = np.zeros((128, WT_COLS), dtype=np.float32)
    wtf[:, WT_C] = c
    for jh in range(2):
        wtf[:, WT_B1 + jh] = b1[jh * 128 + p]
    for oh in range(2):
        wtf[:, WT_VB + oh] = v_b[oh * 128 + p]
    wtf[:, WT_HALFPI] = HALF_PI
    for dh in range(2):
        # vwT[p, dh, o] = v_w[o, dh*128+p]
        wtf[:, WT_VWT + dh * 256 : WT_VWT + (dh + 1) * 256] = v_w[
            :, dh * 128 + p
        ].T
    return wbf.astype(ml_dtypes.bfloat16), wtf


def make_in_maps(inputs):
    x = np.ascontiguousarray(inputs["x"], dtype=np.float32).reshape(N, D, L)
    gg = _grid_rows()
    wbf, wtf = _prep_consts(inputs)
    in_maps = []
    for c in range(NCORES):
        m = {"wbf": wbf, "wtf": wtf}
        # rotate the l-chunks so chunk j holds range (c+j)%NCHUNK
        xc = x[c * NB : (c + 1) * NB].reshape(NB, D, NCHUNK, LC)
        m["x_sh"] = np.ascontiguousarray(
            np.roll(xc, -c, axis=2).reshape(NB, D, L)
        )
        ggc = np.stack(
            [
                gg[:, r * LSH : (r + 1) * LSH]
                for r in [(c + j) % NCORES for j in range(NBRIDGE + 1)]
            ],
            axis=1,
        )  # [2, NBRIDGE+1, LSH]
        m["gg"] = np.ascontiguousarray(ggc).astype(ml_dtypes.bfloat16)
        sel = np.zeros((NCORES, NCHUNK, 128), dtype=np.float32)
        for j in range(NCHUNK):
            sel[(c + j) % NCHUNK, j, :] = 1.0
        m["sel"] = sel.astype(ml_dtypes.bfloat16)
        in_maps.append(m)
    return in_maps


def run(inputs, trace=False):
    nc = _get_nc()
    res = run_bass_kernel_spmd(
        nc, make_in_maps(inputs), core_ids=list(range(NCORES)), trace=trace
    )
    out = np.concatenate([res.results[c]["out"] for c in range(NCORES)], axis=0)
    return out.astype(np.float32), res


def kernel(**inputs) -> np.ndarray:
    out, _ = run(inputs, trace=False)
    return out
